# revision 43
# baseline (speedup 1.0000x reference)
"""GAT layer (nn_GATLayer_88579405512952) — Trainium2 Bass kernel, 8 NeuronCores.

Math (reference):
    Wh  = h @ W                      [N, D]
    Wh1 = Wh @ a[:D],  Wh2 = Wh @ a[D:]
    e[i,j] = leaky_relu(Wh1[i] + Wh2[j], 0.2)       (rank-1 + pointwise)
    out = elu(softmax_row(e) @ Wh)
    (adj is unused by the reference; we never touch it.)

Key algebraic transform used here:
    exp(leaky_relu(s)) = exp(max(s, 0.2 s)) = max(exp(s), exp(0.2 s))
    and softmax rows are invariant to any positive per-row scale, so with
      R1[i] = exp(0.8*Wh1[i]),  E2[j] = exp(Wh2[j]),  E2a[j] = exp(0.2*Wh2[j])
    the unnormalized attention  w'[i,j] = max(R1[i]*E2[j], E2a[j])
    gives exactly softmax(e) after row-normalization. This removes every
    transcendental from the N^2 inner loop: one fused 2-op DVE tensor_scalar
    per [128 x 1024] tile. The row-sum (softmax denominator) is obtained for
    free by augmenting Wh with a ones column inside the PE matmul.

Sharding: each core owns 1024 rows i (flash-attention style 1D row shard),
computes its [1024 x 8192] score block on-chip (never materialized in HBM),
and produces out[c*1024:(c+1)*1024, :]. Wh/E2 are computed redundantly per
core from hT — cheaper than an all-gather (the NRT collective path measures
~40-70us per call here, far more than the redundant compute).

Everything on-chip is bf16 (except PSUM accumulation and the softmax
normalization epilogue, fp32): bf16 keeps the PE at 1 cycle/row even for
the narrow Wh-phase matmuls (fp32r needs a >=256-wide output for that) and
gives the DVE tensor_scalar its fastest mode. Measured rel err ~2.3e-3 vs
the 2e-2 gate.

Host-side marshalling (layout + dtype only; all FLOPs on device): h is
passed transposed (hT, bf16) so the PE can contract over the feature dim,
and the tiny [256,64]@[64,1] param products W@a1, W@a2 are folded into an
augmented weight matrix (constant folding of parameters).

Engine budget per iteration (measured via ablation probes, see probe.py):
PE ~30us busy (26-29 main loop + Wh phase whose narrow matmuls pay serial
LDWEIGHTS), DVE ~30 (64 x ~465ns tensor_scalar: 267ns 4x-mode processing
+ ~200ns fixed per-instruction overhead), Act ~12, DMA ~14 (hidden). DVE
and PE are co-critical, so 5 score tiles late in the loop (t=31,39,...)
are offloaded to the Act engine in relu form: w_act = relu(r1b*e2 - e2a)
drops the +e2a[j] term whose contribution sum_j e2a[j]*v[j,:] is
i-independent; it is re-accumulated in a [65,1] PSUM column (accC) and
added back as a per-partition bias during the epilogue PSUM->SBUF copy
(zero extra cost). Early/dense Act placement stalls PE behind the Act
copy queue and measures SLOWER — placement matters. Tested and rejected:
9 Act tiles (+11us), 3 gpsimd score tiles (+61us — gpsimd is far below
roofline on real HW), gpsimd epilogue offload, collectives (~42-67us per
AllGather via NRT), fp8 double-row matmul (softmax row rescale cannot
fuse into the single DVE pass).

Measured: 18.6-23.4us depending on session (baseline fp32r kernel:
46.6-57.2us measured the same way), rel err 2.3e-3 vs the 2e-2 gate.
"""

import functools

import numpy as np

N = 8192
IN_DIM = 256
OUT_DIM = 64
ALPHA = 0.2
NCORES = 8
ROWS = N // NCORES          # 1024 rows per core
P = 128
JT = N // P                 # 64 j-tiles
KC = IN_DIM // P            # 2 contraction chunks
DA = OUT_DIM + 1            # 65 = [Wh | ones]
EGROUP = 4                  # j-tiles per exp-precompute group
NCH = 8                     # hT DMA chunks
WPOOL_BUFS = 4
EPOOL_BUFS = 2


ACT_EVERY = 8   # every 8th score tile (late in the loop) on the Act engine


def build_nc(repeat: int = 1, ablate: frozenset = frozenset()):
    """Build the Bass program (same NEFF for all 8 cores).

    repeat > 1 re-issues the whole pipeline (DMA included) that many times —
    used by test.py for delta wall-clock timing of the hardware kernel.
    ablate: timing-probe flags that remove parts of the pipeline (results
    become wrong); never set on the production path.
    """
    import concourse.mybir as mybir
    import concourse.tile as tile
    from concourse import bacc
    from concourse.masks import make_identity

    fp32 = mybir.dt.float32
    bf16 = mybir.dt.bfloat16
    Alu = mybir.AluOpType
    Act = mybir.ActivationFunctionType

    nc = bacc.Bacc("TRN2", target_bir_lowering=False, debug=False,
                   num_devices=NCORES)

    hT_d = nc.dram_tensor("hT", [IN_DIM, N], bf16, kind="ExternalInput")
    hTo_d = nc.dram_tensor("hTo", [IN_DIM, ROWS], bf16, kind="ExternalInput")
    waug_d = nc.dram_tensor("waug", [IN_DIM, DA + 3], bf16,
                            kind="ExternalInput")
    out_d = nc.dram_tensor("out", [ROWS, OUT_DIM], fp32, kind="ExternalOutput")

    hT_r = hT_d.ap().rearrange("(c p) j -> p c j", p=P)        # [128, 2, 8192]
    hTo_r = hTo_d.ap().rearrange("(c p) i -> p c i", p=P)      # [128, 2, 1024]
    waug_r = waug_d.ap().rearrange("(c p) d -> p c d", p=P)    # [128, 2, 67]
    out_r = out_d.ap().rearrange("(b p) d -> p b d", p=P)      # [128, 8, 64]

    with tile.TileContext(nc) as tc:
        with (
            tc.tile_pool(name="singles", bufs=1) as singles,
            tc.tile_pool(name="vpool", bufs=1) as vpool,
            tc.tile_pool(name="hpool", bufs=1) as hpool,
            tc.tile_pool(name="wpool", bufs=WPOOL_BUFS) as wpool,
            tc.tile_pool(name="epool", bufs=EPOOL_BUFS) as epool,
            tc.tile_pool(name="ps_wh", bufs=2, space="PSUM") as ps_wh,
            tc.tile_pool(name="ps_acc", bufs=1, space="PSUM") as ps_acc,
            tc.tile_pool(name="ps_misc", bufs=1, space="PSUM") as ps_misc,
            tc.tile_pool(name="ps_tr", bufs=2, space="PSUM") as ps_tr,
        ):
            identity = singles.tile([P, P], fp32)
            make_identity(nc, identity)

            for _rep in range(repeat):
                # ---- load inputs --------------------------------------
                waug_sb = hpool.tile([P, KC, DA + 3], bf16, tag="waug")
                nc.sync.dma_start(waug_sb[:], waug_r)
                hTo_sb = hpool.tile([P, KC, ROWS], bf16, tag="hTo")
                nc.sync.dma_start(hTo_sb[:], hTo_r)
                hT_sb = hpool.tile([P, KC, N], bf16, tag="hT")
                CW = N // NCH
                if "no_hdma" not in ablate:
                    for s in range(NCH):
                        nc.sync.dma_start(
                            hT_sb[:, :, s * CW:(s + 1) * CW],
                            hT_r[:, :, s * CW:(s + 1) * CW],
                        )

                # ---- R1_bcast[p, i] = exp(0.8 * Wh1[i]) for own rows ----
                # Wh1_bcast via matmul with the Wa1 column broadcast to all
                # 128 weight columns -> identical value in every partition.
                wa1_rep = wpool.tile([P, KC, P], bf16, tag="wa1rep")
                _meng = nc.gpsimd if "misc_gps" in ablate else nc.vector
                for c in range(KC):
                    _meng.tensor_copy(
                        wa1_rep[:, c, :],
                        waug_sb[:, c, DA:DA + 1].to_broadcast([P, P]))
                r1b = vpool.tile([P, ROWS], bf16, tag="r1b")
                for half in range(2):
                    ps_bc = ps_misc.tile([P, 512], fp32, tag="misc")
                    sl = slice(half * 512, (half + 1) * 512)
                    for c in range(KC):
                        nc.tensor.matmul(
                            ps_bc[:], wa1_rep[:, c, :], hTo_sb[:, c, sl],
                            start=(c == 0), stop=(c == KC - 1),
                        )
                    nc.scalar.activation(r1b[:, sl], ps_bc[:], Act.Exp,
                                         scale=0.8)

                # ---- Wh phase: V_all[:, t*65:(t+1)*65] = [Wh_t | ones] --
                # bf16 everywhere: PE runs 1 cyc/row even for narrow outputs,
                # and the DVE main-loop tensor_scalar gets the 4x_2p mode.
                # Tiles are batched 4-per-PSUM-buffer so the PSUM->SBUF
                # copies and the Wh2 extraction amortize the per-instruction
                # Act-engine overhead.
                v_all = vpool.tile([P, JT * DA], bf16, tag="v_all")
                v_r = v_all.rearrange("p (t d) -> p t d", d=DA)
                _meng.memset(v_r[:, :, OUT_DIM], 1.0)
                wcols = vpool.tile([P, JT], fp32, tag="wcols")
                e2 = vpool.tile([P, JT], fp32, tag="e2")
                e2a = vpool.tile([P, JT], fp32, tag="e2a")
                e2ab = vpool.tile([P, JT], bf16, tag="e2ab")
                nega = vpool.tile([P, JT], fp32, tag="nega")

                if "act10" in ablate:
                    act_tiles = [t for t in range(JT)
                                 if t >= 16 and t % 5 == 4]
                elif "split8" in ablate:
                    act_tiles = [t for t in range(JT)
                                 if t >= 24 and t % 8 == 7]
                elif ACT_EVERY and "no_split" not in ablate:
                    act_tiles = [t for t in range(JT)
                                 if t >= 24 and t % ACT_EVERY == ACT_EVERY - 1]
                else:
                    act_tiles = []
                gps_tiles = [26, 42, 58] if "gps3" in ablate else []

                WHT = 8 if "wh8" in ablate else JT
                NB = 4
                EB = 16  # e2 exp batch (columns) — keeps main loop fed early
                for g in range(0, WHT, NB):
                    ps4 = ps_wh.tile([P, NB, DA], fp32, tag="wh")
                    for k in range(NB):
                        t = g + k
                        for c in range(KC):
                            nc.tensor.matmul(
                                ps4[:, k, :],
                                hT_sb[:, c, t * P:(t + 1) * P],
                                waug_sb[:, c, 0:DA],
                                start=(c == 0), stop=(c == KC - 1),
                            )
                    nc.scalar.activation(wcols[:, g:g + NB],
                                         ps4[:, :, OUT_DIM], Act.Copy)
                    nc.scalar.activation(v_r[:, g:g + NB, 0:OUT_DIM],
                                         ps4[:, :, 0:OUT_DIM], Act.Copy)
                    done = g + NB
                    if done % EB == 0:
                        e = slice(done - EB, done)
                    elif done >= WHT:
                        e = slice(done - done % EB, done)
                    else:
                        e = None
                    if e is not None:
                        nc.scalar.activation(e2[:, e], wcols[:, e], Act.Exp)
                        nc.scalar.activation(e2a[:, e], wcols[:, e], Act.Exp,
                                             scale=ALPHA)
                        if act_tiles:
                            meng = (nc.gpsimd if "misc_gps" in ablate
                                    else nc.vector)
                            meng.tensor_scalar(e2ab[:, e], e2a[:, e],
                                               1.0, None, Alu.mult)
                            meng.tensor_scalar(nega[:, e], e2a[:, e],
                                               -1.0, None, Alu.mult)

                # ---- main loop: scores + matmul accumulation ------------
                # Score tiles are split between DVE (max form) and the Act
                # engine (relu form): w_act = relu(r1b*e2 - e2a) drops the
                # +e2a[j] term, whose contribution sum_j e2a[j]*v[j,:] is
                # i-independent and accumulated separately in accC, then
                # added back as a per-partition bias during the numt copy.
                acc0 = ps_acc.tile([DA, 512], fp32, tag="acc0")
                acc1 = ps_acc.tile([DA, 512], fp32, tag="acc1")
                if act_tiles:
                    accC = ps_acc.tile([DA, 1], fp32, tag="accC")
                w_prev = None
                for t in range(JT):
                    tt = t % WHT
                    if "half_ts" in ablate and t % 2 == 1:
                        w = w_prev
                    else:
                        w = wpool.tile([P, ROWS], bf16, tag="w")
                        if t in act_tiles:
                            nc.scalar.activation(
                                w[:], r1b[:], Act.Relu,
                                bias=nega[:, tt:tt + 1],
                                scale=e2[:, tt:tt + 1],
                            )
                            nc.tensor.matmul(
                                accC[:], v_r[:, tt, :], e2ab[:, tt:tt + 1],
                                start=(t == act_tiles[0]),
                                stop=(t == act_tiles[-1]),
                            )
                        else:
                            eng = nc.gpsimd if t in gps_tiles else nc.vector
                            eng.tensor_scalar(
                                w[:], r1b[:],
                                e2[:, tt:tt + 1], e2a[:, tt:tt + 1],
                                Alu.mult, Alu.max,
                            )
                        w_prev = w
                    if "no_acc0" not in ablate:
                        nc.tensor.matmul(acc0[:], v_r[:, tt, :], w[:, 0:512],
                                         start=(t == 0), stop=(t == JT - 1))
                    if "no_acc1" not in ablate:
                        nc.tensor.matmul(acc1[:], v_r[:, tt, :], w[:, 512:1024],
                                         start=(t == 0), stop=(t == JT - 1))

                # ---- epilogue: normalize, ELU, transpose, store ---------
                # numt = acc + C, C added as a per-partition bias (free).
                numt = epool.tile([DA, ROWS], fp32, tag="numt")
                if act_tiles:
                    accC_sb = epool.tile([DA, 1], fp32, tag="accCsb")
                    nc.scalar.activation(accC_sb[:], accC[:], Act.Copy)
                    cp = lambda dst, src: nc.scalar.activation(  # noqa: E731
                        dst, src, Act.Identity, bias=accC_sb[:])
                else:
                    cp = lambda dst, src: nc.scalar.activation(  # noqa: E731
                        dst, src, Act.Copy)
                if "no_acc0" in ablate:
                    nc.vector.memset(numt[:, 0:512], 1.0)
                else:
                    cp(numt[:, 0:512], acc0[:])
                if "no_acc1" in ablate:
                    nc.vector.memset(numt[:, 512:1024], 1.0)
                else:
                    cp(numt[:, 512:1024], acc1[:])

                out_all = epool.tile([P, ROWS // P, OUT_DIM], fp32, tag="oall")
                for b in range(0 if "no_epi" in ablate else ROWS // P):
                    ps_t = ps_tr.tile([P, DA], fp32, tag="tr", name="ps_t")
                    nc.tensor.transpose(ps_t[:], numt[:, b * P:(b + 1) * P],
                                        identity[0:DA, 0:DA])
                    zinv = wpool.tile([P, 1], fp32, tag="zinv")
                    nc.vector.reciprocal(zinv[:], ps_t[:, OUT_DIM:DA])
                    nc.vector.tensor_scalar(
                        out_all[:, b, :], ps_t[:, 0:OUT_DIM], zinv[:], None,
                        Alu.mult,
                    )

                # ELU, exactly: (max(x,0) - 1) + exp(min(x,0))
                flat = out_all.rearrange("p b d -> p (b d)")
                if "no_epi" in ablate:
                    nc.vector.memset(flat, 0.0)
                else:
                    eng = nc.gpsimd if "elu_gps" in ablate else nc.vector
                    r = epool.tile([P, ROWS // P * OUT_DIM], fp32, tag="elur")
                    m = epool.tile([P, ROWS // P * OUT_DIM], fp32, tag="elum")
                    eng.tensor_scalar(r[:], flat, 0.0, -1.0,
                                      Alu.max, Alu.add)
                    eng.tensor_scalar(m[:], flat, 0.0, None, Alu.min)
                    nc.scalar.activation(m[:], m[:], Act.Exp)
                    eng.tensor_tensor(flat, r[:], m[:], Alu.add)

                nc.sync.dma_start(out_r, out_all[:])

    nc.compile()
    return nc


@functools.lru_cache(maxsize=8)
def _cached_nc(repeat: int = 1, ablate: frozenset = frozenset()):
    return build_nc(repeat, ablate)


class _Runner:
    """Compile once, load once, execute many times on the 8 cores.

    Mirrors concourse.bass2jax.run_bass_via_pjrt's multi-core path but caches
    the jitted executable and the device-resident inputs, so repeated calls
    measure (dispatch + device execution) only.  Output tensors are fully
    written by the kernel, so the zero "donation" buffers are passed as
    ordinary (cached) params without donation.
    """

    def __init__(self, repeat: int = 1, ablate: frozenset = frozenset()):
        import jax
        from jax.experimental.shard_map import shard_map
        from jax.sharding import Mesh, NamedSharding, PartitionSpec
        import concourse.mybir as mybir
        from concourse import bass2jax

        self.jax = jax
        nc = _cached_nc(repeat, ablate)
        partition_name = (nc.partition_id_tensor.name
                          if nc.partition_id_tensor else None)
        bass2jax.install_neuronx_cc_hook()

        in_names, out_names, out_avals, zero_outs = [], [], [], []
        for alloc in nc.m.functions[0].allocations:
            if not isinstance(alloc, mybir.MemoryLocationSet):
                continue
            name = alloc.memorylocations[0].name
            if alloc.kind == "ExternalInput":
                if name != partition_name:
                    in_names.append(name)
            elif alloc.kind == "ExternalOutput":
                shape = tuple(alloc.tensor_shape)
                dt = mybir.dt.np(alloc.dtype)
                out_names.append(name)
                out_avals.append(jax.core.ShapedArray(shape, dt))
                zero_outs.append(np.zeros((NCORES * shape[0], *shape[1:]), dt))
        self.in_names = in_names
        self.out_names = out_names
        self.out_shapes = [tuple(a.shape) for a in out_avals]
        all_names = tuple(in_names + out_names)
        if partition_name is not None:
            all_names = all_names + (partition_name,)

        def _body(*args):
            operands = list(args)
            if partition_name is not None:
                operands.append(bass2jax.partition_id_tensor())
            outs = bass2jax._bass_exec_p.bind(
                *operands,
                out_avals=tuple(out_avals),
                in_names=all_names,
                out_names=tuple(out_names),
                lowering_input_output_aliases=(),
                sim_require_finite=True,
                sim_require_nnan=True,
                nc=nc,
            )
            return tuple(outs)

        devices = jax.devices()[:NCORES]
        mesh = Mesh(np.asarray(devices), ("core",))
        n_args = len(in_names) + len(out_names)
        self.fn = jax.jit(
            shard_map(
                _body, mesh=mesh,
                in_specs=(PartitionSpec("core"),) * n_args,
                out_specs=(PartitionSpec("core"),) * len(out_names),
                check_rep=False,
            ),
            keep_unused=True,
        )
        self.sharding = NamedSharding(mesh, PartitionSpec("core"))
        self.zero_dev = [jax.device_put(z, self.sharding) for z in zero_outs]
        self.dev_inputs = None
        self._inputs_key = None

    def set_inputs(self, in_maps):
        key = id(in_maps)
        if self._inputs_key == key and self.dev_inputs is not None:
            return
        concat = [
            np.concatenate([np.asarray(m[name]) for m in in_maps], axis=0)
            for name in self.in_names
        ]
        self.dev_inputs = [
            self.jax.device_put(c, self.sharding) for c in concat
        ]
        self.jax.block_until_ready(self.dev_inputs)
        self._inputs_key = key

    def execute(self):
        outs = self.fn(*self.dev_inputs, *self.zero_dev)
        self.jax.block_until_ready(outs)
        return outs

    def results(self):
        outs = self.execute()
        per_core = []
        for c in range(NCORES):
            per_core.append({
                name: np.asarray(outs[i]).reshape(
                    NCORES, *self.out_shapes[i])[c]
                for i, name in enumerate(self.out_names)
            })
        return per_core


@functools.lru_cache(maxsize=8)
def _cached_runner(repeat: int = 1, ablate: frozenset = frozenset()):
    return _Runner(repeat, ablate)


def _marshal(h, W, a):
    import ml_dtypes
    bf16 = ml_dtypes.bfloat16
    h = np.asarray(h, dtype=np.float32)
    W = np.asarray(W, dtype=np.float32)
    a = np.asarray(a, dtype=np.float32).reshape(2 * OUT_DIM, 1)
    hT = np.ascontiguousarray(h.T).astype(bf16)        # [256, 8192]
    wa1 = W @ a[:OUT_DIM]                              # [256, 1]
    wa2 = W @ a[OUT_DIM:]                              # [256, 1]
    # col layout: [W(0:64) | wa2(64) | wa1(65) | pad] so the Wh-phase matmul
    # reads the contiguous 65-col [W|wa2] slice and wa1 sits at col DA.
    waug = np.ascontiguousarray(
        np.concatenate([W, wa2, wa1, np.zeros((IN_DIM, 2), np.float32)],
                       axis=1)).astype(bf16)           # [256, 68]
    in_maps = []
    for c in range(NCORES):
        in_maps.append({
            "hT": hT,
            "hTo": np.ascontiguousarray(hT[:, c * ROWS:(c + 1) * ROWS]),
            "waug": waug,
        })
    return in_maps


def run_on_cores(in_maps, repeat: int = 1):
    runner = _cached_runner(repeat)
    runner.set_inputs(in_maps)
    return runner.results()


def _run_fallback(in_maps):
    """Slow-but-blessed execution path (fresh compile each call)."""
    from concourse.bass_utils import run_bass_kernel_spmd
    nc = build_nc(1)
    res = run_bass_kernel_spmd(nc, in_maps, core_ids=list(range(NCORES)))
    return res.results


def kernel(h, adj, W, a):
    import time
    in_maps = _marshal(h, W, a)
    res = None
    last_exc = None
    for attempt in range(4):
        try:
            if attempt < 3:
                res = run_on_cores(in_maps, repeat=1)
            else:
                res = _run_fallback(in_maps)
            break
        except Exception as e:  # device wedge etc: wait for recovery, retry
            last_exc = e
            _cached_runner.cache_clear()
            _cached_nc.cache_clear()
            time.sleep(20 * (attempt + 1))
    if res is None:
        raise last_exc
    out = np.concatenate([r["out"] for r in res], axis=0)
    return out.astype(np.float32)


if __name__ == "__main__":
    rng = np.random.default_rng(0)
    h = rng.standard_normal((N, IN_DIM), dtype=np.float32)
    W = (rng.standard_normal((IN_DIM, OUT_DIM), dtype=np.float32) * 0.1)
    a = (rng.standard_normal((2 * OUT_DIM, 1), dtype=np.float32) * 0.1)
    adj = np.zeros((N, N), dtype=bool)
    out = kernel(h, adj, W, a)
    print("out", out.shape, out.dtype, float(out.mean()))

